# revision 7
# baseline (speedup 1.0000x reference)
"""Trainium2 Bass kernel for nn_LongTermAttention (continuous softmax readout).

Math (per query row i, basis j):
    sigma_sq_i = -0.5 / theta[i,1];  mu_i = theta[i,0] * sigma_sq_i
    s2[i,j]    = basis_sigma[j]^2 + sigma_sq_i
    r[i,j]     = (1/sqrt(2pi)) * exp(-0.5*(mu_i-basis_mu[j])^2/s2) / sqrt(s2)
               = exp(-0.5*((mu_i-bmu_j)^2/s2 + ln s2) + lnC)
    out        = r @ Bv        # [N, D]

Sharding: data-parallel over N across 8 cores (N_loc = N/8 rows per core).
basis params + Bv replicated. On-chip layout: r is computed TRANSPOSED
(basis j on partitions, rows i on free dim) so each [128j, 128i] slice is
directly the stationary lhsT operand of the PE matmul (contraction over j),
with Bv [j, d] as the moving operand. r and Bv are cast to bf16 for the
matmul; everything else is fp32.

ACT uses only Square / Ln / Exp / Copy -> one table set
(natural_log_exp_and_others), no table-switch cost.
"""

import math
import numpy as np

import concourse.bass as bass
import concourse.mybir as mybir
import concourse.tile as tile
from concourse import bacc
from concourse.bass_utils import run_bass_kernel_spmd

F32 = mybir.dt.float32
BF16 = mybir.dt.bfloat16

N_CORES = 8
N = 65536
NB = 1024
D = 1024
N_LOC = N // N_CORES          # 8192 rows per core

LN_C = float(math.log(1.0 / math.sqrt(2.0 * math.pi)))

# tunables
IC = 1024                     # rows per i-chunk
USE_DIVIDE = False            # DVE tensor_tensor divide is not supported by walrus ISA


def _bcast_ap(src: bass.AP, parts: int = 128) -> bass.AP:
    """Replicate a DRAM row vector across `parts` partitions (step-0 DMA)."""
    return bass.AP(tensor=src.tensor, offset=src.offset, ap=[[0, parts]] + list(src.ap))


def build_program(n_loc: int = N_LOC, nb: int = NB, d: int = D, ic: int = IC):
    nc = bacc.Bacc("TRN2", target_bir_lowering=False, debug=False)

    theta = nc.declare_dram_parameter("theta", [n_loc, 2], F32, isOutput=False)
    basis_mu = nc.declare_dram_parameter("basis_mu", [nb], F32, isOutput=False)
    basis_sigma = nc.declare_dram_parameter("basis_sigma", [nb], F32, isOutput=False)
    bv = nc.declare_dram_parameter("Bv", [nb, d], F32, isOutput=False)
    out = nc.declare_dram_parameter("out", [n_loc, d], F32, isOutput=True)

    mu_scr = nc.dram_tensor("mu_scratch", [n_loc], F32)
    ssq_scr = nc.dram_tensor("ssq_scratch", [n_loc], F32)

    n_jb = nb // 128            # basis chunks (partition dim)
    n_ic = n_loc // ic          # i-chunks
    n_m = ic // 128             # 128-row subtiles per i-chunk
    n_d = d // 512              # 512-wide output column chunks
    tcols = n_loc // 128        # free cols per partition in row-param layout

    with tile.TileContext(nc) as tc:
        with (
            tc.tile_pool(name="consts", bufs=1) as consts,
            tc.tile_pool(name="stage", bufs=2) as stage,
            tc.tile_pool(name="bc", bufs=4) as bcp,
            tc.tile_pool(name="temps", bufs=2) as temps,
            tc.tile_pool(name="rt", bufs=2 * n_jb) as rtp,
            tc.tile_pool(name="ctx", bufs=8) as ctxp,
            tc.tile_pool(name="psum", bufs=8, space="PSUM") as psum,
        ):
            # ---- per-row params: ssq/mu in [128, tcols] layout, row i = p*tcols + t
            th = consts.tile([128, tcols, 2], F32)
            nc.sync.dma_start(out=th, in_=theta.ap().rearrange("(p t) c -> p t c", p=128))
            th1n = consts.tile([128, tcols], F32)
            nc.vector.tensor_scalar(th1n, th[:, :, 1], -2.0, None, mybir.AluOpType.mult)
            ssq64 = consts.tile([128, tcols], F32)
            nc.vector.reciprocal_approx_fast(ssq64, th1n)     # = -0.5/theta1 = sigma_sq
            mu64 = consts.tile([128, tcols], F32)
            nc.vector.tensor_tensor(mu64, th[:, :, 0], ssq64, mybir.AluOpType.mult)
            nc.sync.dma_start(out=mu_scr.ap().rearrange("(p t) -> p t", p=128), in_=mu64)
            nc.sync.dma_start(out=ssq_scr.ap().rearrange("(p t) -> p t", p=128), in_=ssq64)

            # ---- basis constants: [128, n_jb] column-per-chunk layout
            bmu_sb = consts.tile([128, n_jb], F32)
            nc.sync.dma_start(out=bmu_sb, in_=basis_mu.ap().rearrange("(b p) -> p b", p=128))
            neg_bmu = consts.tile([128, n_jb], F32)
            nc.vector.tensor_scalar(neg_bmu, bmu_sb, -1.0, None, mybir.AluOpType.mult)
            bsig_sb = consts.tile([128, n_jb], F32)
            nc.sync.dma_start(out=bsig_sb, in_=basis_sigma.ap().rearrange("(b p) -> p b", p=128))
            bsig2 = consts.tile([128, n_jb], F32)
            nc.vector.tensor_tensor(bsig2, bsig_sb, bsig_sb, mybir.AluOpType.mult)
            lnc_sb = consts.tile([128, 1], F32)
            nc.vector.memset(lnc_sb, LN_C)

            # ---- Bv -> bf16 tiles [128, d] per basis chunk
            bv_t = []
            for jb in range(n_jb):
                stg = stage.tile([128, d], F32, tag="bvstage")
                nc.sync.dma_start(out=stg, in_=bv.ap()[jb * 128:(jb + 1) * 128, :])
                bvt = consts.tile([128, d], BF16, tag=f"bv{jb}")
                nc.vector.tensor_copy(bvt, stg)
                bv_t.append(bvt)

            # ---- main loop over i-chunks
            for c in range(n_ic):
                bc_mu = bcp.tile([128, ic], F32, tag="bc_mu")
                nc.sync.dma_start(out=bc_mu, in_=_bcast_ap(mu_scr.ap()[c * ic:(c + 1) * ic]))
                bc_ssq = bcp.tile([128, ic], F32, tag="bc_ssq")
                nc.sync.dma_start(out=bc_ssq, in_=_bcast_ap(ssq_scr.ap()[c * ic:(c + 1) * ic]))

                rts = []
                for jb in range(n_jb):
                    s2 = temps.tile([128, ic], F32, tag="s2")
                    nc.vector.tensor_scalar(s2, bc_ssq, bsig2[:, jb:jb + 1], None,
                                            mybir.AluOpType.add)
                    t2 = temps.tile([128, ic], F32, tag="t2")
                    nc.scalar.activation(t2, bc_mu, mybir.ActivationFunctionType.Square,
                                         bias=neg_bmu[:, jb:jb + 1])
                    lns2 = temps.tile([128, ic], F32, tag="lns2")
                    nc.scalar.activation(lns2, s2, mybir.ActivationFunctionType.Ln)
                    ratio = temps.tile([128, ic], F32, tag="ratio")
                    if USE_DIVIDE:
                        nc.vector.tensor_tensor(ratio, t2, s2, mybir.AluOpType.divide)
                    else:
                        u = temps.tile([128, ic], F32, tag="u")
                        nc.vector.reciprocal_approx_fast(u, s2)
                        nc.vector.tensor_tensor(ratio, t2, u, mybir.AluOpType.mult)
                    sm = temps.tile([128, ic], F32, tag="sm")
                    nc.vector.tensor_tensor(sm, ratio, lns2, mybir.AluOpType.add)
                    rt = rtp.tile([128, ic], BF16, tag="rt")
                    nc.scalar.activation(rt, sm, mybir.ActivationFunctionType.Exp,
                                         bias=lnc_sb[:], scale=-0.5)
                    rts.append(rt)

                for m in range(n_m):
                    for dd in range(n_d):
                        pt = psum.tile([128, 512], F32, tag="pt")
                        for jb in range(n_jb):
                            nc.tensor.matmul(pt, rts[jb][:, m * 128:(m + 1) * 128],
                                             bv_t[jb][:, dd * 512:(dd + 1) * 512],
                                             start=(jb == 0), stop=(jb == n_jb - 1))
                        cs = ctxp.tile([128, 512], F32, tag="cs")
                        nc.any.tensor_copy(cs, pt)
                        r0 = c * ic + m * 128
                        nc.sync.dma_start(
                            out=out.ap()[r0:r0 + 128, dd * 512:(dd + 1) * 512], in_=cs)
    nc.compile()
    return nc


_PROGRAM_CACHE: dict = {}


def _get_program():
    if "main" not in _PROGRAM_CACHE:
        _PROGRAM_CACHE["main"] = build_program()
    return _PROGRAM_CACHE["main"]


def run(inputs: dict, trace: bool = False):
    theta = np.ascontiguousarray(inputs["theta"], dtype=np.float32)
    basis_mu = np.ascontiguousarray(inputs["basis_mu"], dtype=np.float32)
    basis_sigma = np.ascontiguousarray(inputs["basis_sigma"], dtype=np.float32)
    bv = np.ascontiguousarray(inputs["Bv"], dtype=np.float32)

    nc = _get_program()
    shards = np.split(theta, N_CORES, axis=0)
    in_maps = [
        {"theta": shards[i], "basis_mu": basis_mu, "basis_sigma": basis_sigma, "Bv": bv}
        for i in range(N_CORES)
    ]
    res = run_bass_kernel_spmd(nc, in_maps, list(range(N_CORES)), trace=trace)
    full = np.concatenate([res.results[i]["out"] for i in range(N_CORES)], axis=0)
    return full, res


def kernel(**inputs) -> np.ndarray:
    full, _ = run(inputs, trace=False)
    return full
